# revision 30
# baseline (speedup 1.0000x reference)
"""Trainium2 Bass kernel for entmax-1.5 via Newton + exact correction
(nn_EntmaxNsect).

Full input: X [4096, 32000] f32 -> entmax weights, same shape.
Data-parallel over 8 NeuronCores (512 rows each), row-blocks of 128.

Math (validated offline vs the jax reference, rel err ~1.7e-3 vs 2e-2 gate):
  alpha=1.5 -> p = relu(0.5x - tau)^2 / Z with tau the root of
  f(tau) = sum relu(0.5x - tau)^2 - 1. Newton on window-16 chunk maxima
  (coarse iters on window-128 maxima, fine on window-16) converges th to
  ~4e-3 of the root; one exact full-data Newton step lands tau within
  ~2e-4, well within what the reference's renormalization washes out.
  Z from the exact quadratic expansion around th; rsqrt(Z) via one NR
  iteration (Z ~= 1 always). Output p = Square(sv*0.5*y - dd*sv) where
  y = relu(x - th2) -- y is already relu'd at th, and skipping the relu at
  tau only perturbs entries by <= (dd*sv)^2 ~ 2e-5.

Schedule: x is loaded f32->f16 (SWDGE cast DMA), halving SBUF so TWO full
row-blocks fit -> cross-block double buffering with no load/compute stall.
The eval pass transforms x -> y in place (DVE, 4x f16 mode); eval squares
(exact sum A) are split between ACT and DVE; the output pass is a single
ACT Square reading y, written f16 and cast f16->f32 on the store DMA.
Pool issues all DMAs (SWDGE casts). Ladder Newton iterations run in
th2 = 2*th space to save ops; clamps use two-scalar tensor_scalar ops.
"""

import contextlib
import numpy as np


def _bc(small, like):
    from concourse.bass import broadcast_tensor_aps
    return broadcast_tensor_aps(small, like)[0]

P = 128
D = 32000
W16 = 16
M16 = D // W16        # 2000
W8 = 8
M128 = M16 // W8      # 250
NQ = 4
QW = D // NQ          # 8000
NCH = 16
CW = D // NCH         # 2000
NA = 13               # eval-square chunks done by ACT; rest by DVE
NOC = 4
OW = D // NOC         # 8000
NO2 = 3
COARSE_ITERS = 4
FINE_ITERS = 2


def build_entmax_kernel(nc, n_rows, reps=1):
    import concourse.mybir as mybir

    f32 = mybir.dt.float32
    f16 = mybir.dt.float16
    AX = mybir.AxisListType.X
    OP = mybir.AluOpType
    AF = mybir.ActivationFunctionType

    x = nc.dram_tensor("X", [n_rows, D], f32, kind="ExternalInput")
    out = nc.dram_tensor("OUT", [n_rows, D], f32, kind="ExternalOutput")

    nblk_real = n_rows // P
    nblk = nblk_real * reps
    assert n_rows % P == 0

    def rowbase(b):
        return (b % nblk_real) * P

    ctx = contextlib.ExitStack()
    with ctx:
        def sb(name, shape, dt=f32):
            return ctx.enter_context(nc.sbuf_tensor(name, shape, dt))

        xq = [sb(f"xq{i}", [P, D], f16) for i in range(2)]
        o2 = [sb(f"o2_{i}", [P, OW], f16) for i in range(NO2)]
        m16 = sb("m16", [P, M16], f16)
        fold4 = sb("fold4", [P, QW // 2], f16)
        # rotating square scratches (2 per engine so back-to-back square
        # ops never WAW the same buffer without a sync edge)
        sqA = [sb(f"sqA{i}", [P, CW], f16) for i in range(2)]
        sqD = [sb(f"sqD{i}", [P, CW], f16) for i in range(2)]
        l250 = sb("l250", [P, M128])
        m128 = sb("m128", [P, M128])
        Bp = sb("Bp", [P, NCH])
        Ap = sb("Ap", [P, NCH])
        (mxx, lo02, th, th2, lB, lF, lnum, lden, lrec,
         Bx, Axm, Fs, rr, tau, dd, negBx, zt, Z, y1, u, sv) = (
            sb(n, [P, 1]) for n in (
                "mxx", "lo02", "th", "th2", "lB", "lF", "lnum",
                "lden", "lrec", "Bx", "Axm", "Fs", "rr", "tau", "dd",
                "negBx", "zt", "Z", "y1", "u", "sv"))
        sc_t = sb("sc_t", [P, 1])
        bi_t = sb("bi_t", [P, 1])

        s_ld = [ctx.enter_context(nc.semaphore(f"s_ld{q}"))
                for q in range(NQ)]
        s_y = ctx.enter_context(nc.semaphore("s_y"))
        s_sq = ctx.enter_context(nc.semaphore("s_sq"))
        s_dsq = ctx.enter_context(nc.semaphore("s_dsq"))
        s_tau = ctx.enter_context(nc.semaphore("s_tau"))
        s_oc = ctx.enter_context(nc.semaphore("s_oc"))
        s_st = [ctx.enter_context(nc.semaphore(f"s_st{j}"))
                for j in range(NO2)]

        block = ctx.enter_context(nc.Block())

        @block.gpsimd
        def _(gp):
            for b in range(nblk):
                buf = b % 2
                r0 = rowbase(b)
                for q in range(NQ):
                    if b >= 2:
                        # xq[buf] held block b-2's y; quarter q freed when
                        # OUT_{b-2} chunk q (OW == QW) is written.
                        gp.wait_ge(s_oc, NOC * (b - 2) + q + 1)
                    gp.dma_start(
                        xq[buf][:, q * QW:(q + 1) * QW],
                        x[r0:r0 + P, q * QW:(q + 1) * QW],
                    ).then_inc(s_ld[q], 16)
                if b >= 1:
                    rp = rowbase(b - 1)
                    for oc in range(NOC):
                        g = NOC * (b - 1) + oc
                        gp.wait_ge(s_oc, g + 1)
                        gp.dma_start(
                            out[rp:rp + P, oc * OW:(oc + 1) * OW],
                            o2[g % NO2][:],
                        ).then_inc(s_st[g % NO2], 16)
            rp = rowbase(nblk - 1)
            for oc in range(NOC):
                g = NOC * (nblk - 1) + oc
                gp.wait_ge(s_oc, g + 1)
                gp.dma_start(
                    out[rp:rp + P, oc * OW:(oc + 1) * OW], o2[g % NO2][:]
                ).then_inc(s_st[g % NO2], 16)

        @block.vector
        def _(dve):
            d = dve.drain
            for b in range(nblk):
                buf = b % 2
                # "window" maxima via strided fold trees (tensor_tensor max
                # at f16 2x beats the 1x-only tensor_reduce). The grouping
                # is an arbitrary fixed partition of each row into 16-sets,
                # which is all the chunk-max Newton ladder needs.
                npq = M16 // NQ
                for q in range(NQ):
                    dve.wait_ge(s_ld[q], 16 * (b + 1))
                    xqv = xq[buf][:, q * QW:(q + 1) * QW]
                    dve.tensor_tensor(fold4[:, 0:4000], xqv[:, 0:4000],
                                      xqv[:, 4000:8000], op=OP.max)
                    d()
                    dve.tensor_tensor(fold4[:, 0:2000], fold4[:, 0:2000],
                                      fold4[:, 2000:4000], op=OP.max)
                    d()
                    dve.tensor_tensor(fold4[:, 0:1000], fold4[:, 0:1000],
                                      fold4[:, 1000:2000], op=OP.max)
                    d()
                    dve.tensor_tensor(
                        m16[:, q * npq:(q + 1) * npq], fold4[:, 0:500],
                        fold4[:, 500:1000], op=OP.max)
                    d()
                # m128: window-8 fold over m16
                dve.tensor_tensor(fold4[:, 0:1000], m16[:, 0:1000],
                                  m16[:, 1000:2000], op=OP.max)
                d()
                dve.tensor_tensor(fold4[:, 0:500], fold4[:, 0:500],
                                  fold4[:, 500:1000], op=OP.max)
                d()
                dve.tensor_tensor(m128[:], fold4[:, 0:250],
                                  fold4[:, 250:500], op=OP.max)
                d()
                dve.tensor_scalar(l250[:], m128[:], 0.0, None,
                                  op0=OP.add, op1=OP.max, accum_out=mxx[:])
                d()
                # th2-space: th2 = 2*th; bracket is [mxx-2, mxx]
                dve.tensor_scalar_add(th2[:], mxx[:], -1.0)
                dve.tensor_scalar_add(lo02[:], mxx[:], -2.0)
                d()

                def newton_update(clamp, last=False):
                    # th2 += 2 * (F/4 - 1) / B  =  (F/2 - 2) / B
                    dve.tensor_scalar(lnum[:], lF[:], 0.5, -2.0,
                                      op0=OP.mult, op1=OP.add)
                    dve.tensor_scalar_max(lden[:], lB[:], 1e-20)
                    d()
                    dve.reciprocal(lrec[:], lden[:])
                    d()
                    dve.scalar_tensor_tensor(
                        th2[:], lnum[:], lrec[:], th2[:],
                        op0=OP.mult, op1=OP.add)
                    d()
                    if clamp:
                        dve.tensor_scalar(th2[:], th2[:], lo02[:], mxx[:],
                                          op0=OP.max, op1=OP.min)
                        d()

                for it in range(COARSE_ITERS):
                    dve.scalar_tensor_tensor(
                        l250[:], m128[:], th2[:], _bc(th2[:], l250[:]),
                        op0=OP.max, op1=OP.subtract, accum_out=lB[:])
                    d()
                    dve.scalar_tensor_tensor(
                        l250[:], l250[:], 1.0, l250[:],
                        op0=OP.mult, op1=OP.mult, accum_out=lF[:])
                    d()
                    newton_update(clamp=(it == COARSE_ITERS - 1))
                for it in range(FINE_ITERS):
                    dve.scalar_tensor_tensor(
                        sqD[0][:], m16[:], th2[:], _bc(th2[:], sqD[0][:]),
                        op0=OP.max, op1=OP.subtract, accum_out=lB[:])
                    d()
                    dve.scalar_tensor_tensor(
                        sqD[0][:], sqD[0][:], 1.0, sqD[0][:],
                        op0=OP.mult, op1=OP.mult, accum_out=lF[:])
                    d()
                    newton_update(clamp=True)

                # eval pass 1: x -> y = relu(x - th2) in place, one 4x
                # tensor_scalar per chunk
                for c in range(NCH):
                    xc = xq[buf][:, c * CW:(c + 1) * CW]
                    dve.tensor_scalar(
                        xc, xc, th2[:], 0.0, op0=OP.subtract, op1=OP.max,
                    ).then_inc(s_y, 1)
                d()
                # eval pass 2: exact B per chunk (4x add-accum into a
                # rotating scratch), then DVE's share of the exact squares.
                # One rotation counter paces WAW on the two scratches.
                nj = 2 * NCH - NA
                for c in range(NCH):
                    j = nj * b + c
                    if c >= 2:
                        dve.wait_ge(s_dsq, j - 1)
                    dve.tensor_scalar(
                        sqD[c % 2][:], xq[buf][:, c * CW:(c + 1) * CW],
                        0.0, None, op0=OP.add, op1=OP.add,
                        accum_out=Bp[:, c:c + 1],
                    ).then_inc(s_dsq, 1)
                for c in range(NA, NCH):
                    j = nj * b + NCH + (c - NA)
                    dve.wait_ge(s_dsq, j - 1)
                    dve.scalar_tensor_tensor(
                        sqD[j % 2][:], xq[buf][:, c * CW:(c + 1) * CW], 1.0,
                        xq[buf][:, c * CW:(c + 1) * CW],
                        op0=OP.mult, op1=OP.mult, accum_out=Ap[:, c:c + 1],
                    ).then_inc(s_dsq, 1)
                d()
                dve.tensor_reduce(Bx[:], Bp[:], axis=AX, op=OP.add)
                d()
                dve.tensor_scalar_max(lden[:], Bx[:], 1e-20)
                dve.tensor_scalar_mul(negBx[:], Bx[:], -0.5)
                d()
                dve.reciprocal(lrec[:], lden[:])
                d()

                # tail (all in th2 = 2*th space):
                #   rr2 = th2 + 2*f/Bx, tau2 = clamp(rr2), dd2 = tau2 - th2
                #   Z = 1 + f - (dd2/2)*Bx, sv = rsqrt(Z) via one NR iter
                #   out = (sv/2 * y - dd2/2*sv)^2  (ACT Square scale/bias)
                dve.wait_ge(s_sq, NA * (b + 1))
                dve.tensor_reduce(Axm[:], Ap[:], axis=AX, op=OP.add)
                d()
                dve.tensor_scalar(Fs[:], Axm[:], 0.25, -1.0,
                                  op0=OP.mult, op1=OP.add)
                dve.tensor_scalar(lnum[:], Axm[:], 0.5, -2.0,
                                  op0=OP.mult, op1=OP.add)
                d()
                dve.scalar_tensor_tensor(
                    rr[:], lnum[:], lrec[:], th2[:], op0=OP.mult, op1=OP.add)
                d()
                dve.tensor_scalar(tau[:], rr[:], lo02[:], mxx[:],
                                  op0=OP.max, op1=OP.min)
                d()
                dve.tensor_sub(dd[:], tau[:], th2[:])
                d()
                dve.scalar_tensor_tensor(
                    zt[:], dd[:], negBx[:], Fs[:], op0=OP.mult, op1=OP.add)
                d()
                dve.tensor_scalar(Z[:], zt[:], 1.0, 0.5,
                                  op0=OP.add, op1=OP.max)
                d()
                dve.tensor_scalar(y1[:], Z[:], -0.5, 1.5,
                                  op0=OP.mult, op1=OP.add)
                d()
                dve.scalar_tensor_tensor(
                    u[:], y1[:], y1[:], Z[:], op0=OP.mult, op1=OP.mult)
                d()
                dve.tensor_scalar(u[:], u[:], -0.5, 1.5,
                                  op0=OP.mult, op1=OP.add)
                d()
                dve.tensor_mul(sv[:], y1[:], u[:])
                d()
                dve.tensor_scalar_mul(sc_t[:], sv[:], 0.5)
                dve.tensor_scalar(bi_t[:], dd[:], sv[:], -0.5,
                                  op0=OP.mult, op1=OP.mult
                                  ).then_inc(s_tau, 1)

        @block.scalar
        def _(act):
            for b in range(nblk):
                buf = b % 2
                for c in range(NA):
                    act.wait_ge(s_y, NCH * b + c + 1)
                    if c >= 2:
                        act.wait_ge(s_sq, NA * b + c - 1)
                    act.activation(
                        sqA[c % 2][:], xq[buf][:, c * CW:(c + 1) * CW],
                        AF.Square, accum_out=Ap[:, c:c + 1],
                    ).then_inc(s_sq, 1)
                act.wait_ge(s_tau, b + 1)
                for oc in range(NOC):
                    g = NOC * b + oc
                    if g >= NO2:
                        act.wait_ge(s_st[g % NO2], 16 * ((g - g % NO2) // NO2))
                    act.activation(
                        o2[g % NO2][:], xq[buf][:, oc * OW:(oc + 1) * OW],
                        AF.Square, bias=bi_t[:], scale=sc_t[:],
                    ).then_inc(s_oc, 1)

    return nc


_CACHE = {}


def _get_nc(n_rows, reps=1):
    key = (n_rows, reps)
    if key in _CACHE:
        return _CACHE[key]
    import concourse.bass as bass

    nc = bass.Bass("TRN2")
    build_entmax_kernel(nc, n_rows, reps)
    nc.finalize()
    _CACHE[key] = nc
    return nc


def kernel(X: np.ndarray) -> np.ndarray:
    from concourse.bass_utils import run_bass_kernel_spmd

    n_cores = 8
    rows = X.shape[0]
    shard = rows // n_cores
    X = np.ascontiguousarray(X, dtype=np.float32)
    nc = _get_nc(shard)
    in_maps = [{"X": X[i * shard:(i + 1) * shard]} for i in range(n_cores)]
    res = run_bass_kernel_spmd(nc, in_maps, core_ids=list(range(n_cores)))
    return np.concatenate([r["OUT"] for r in res.results], axis=0)
